# revision 1
# baseline (speedup 1.0000x reference)
"""Trainium2 Bass kernel for nn_BandwidthPredictorNNHall.

Math: for each batch b (8 of them, one per NeuronCore) with particles
x [n=1024, d=4]:
    pilot_d = 1.0592 * std(x_d, ddof=1) * n^(-1/8)
    q = x / pilot,   K_ij = exp(-0.5 * |q_i - q_j|^2)
    s2_d = sum_ij K_ij ((q_jd - q_id)^2 - 1)
    s3_d = sum_ij K_ij (dx^3 - 3 dx)  == 0 exactly (odd under i<->j swap),
           so bandwidth2 is fp-cancellation noise in the reference
           (|bw2/bw1| ~ 6e-9) and is treated as 0.
With Mp = [1, p_1..p_4, p_1^2..p_4^2] (n x 9, RAW particle units), every sum
needed for s2 is an entry of V = Mp^T K Mp after a host-side 1/pilot^2
rescale:
    s2_d = ((V[0,5+d] + V[5+d,0] - 2 V[1+d,1+d]) / pilot_d^2 - V[0,0]) / sqrt(2pi)
The device computes V (9x9) and var (4) per batch; the host applies the
final ~30 scalar flops per batch.

Device pipeline per core (engine-balance driven; ScalarE's 1M exps are the
floor, everything else hides behind or around them):
  - One input DMA (each dma_start costs ~0.6us of queue time plus ~1.5us
    latency): a 3D-strided load mstatall [128, 8(tile), 4] in particle-major
    layout. The feature-major Gram operands are built from it with 8 small
    PE transposes instead of a second (slow, 4-byte-run) strided DMA.
  - sum(p) and sum(p^2) accumulate on the PE as two sequential matmul
    groups against a ones vector; tiny PE transposes move the results into
    row form. var/pinv2 = 1/(FACT^2 var) needs only a reciprocal -- no
    sqrt, so ScalarE runs just {Exp, Copy}: one activation-table set, one
    LoadActFuncSet, and it overlaps the DMA latency.
  - G_ij = q_i . q_j is computed as sum_d (p_id/pilot_d^2) p_jd (float32r
    streams at 1 cycle/row; even bf16-coarse rounding would move the final
    output < 1.5e-4, far below the reference's own fp32 noise).
  - K'' = exp(G - r_i/2): one [128,1024] ScalarE activation per row tile
    with per-partition bias, reading a 2-bank PSUM tile. r_i comes from a
    multiply + negated reduce against a PE-broadcast 0.5/pilot^2 row.
  - K'' is the true K column-scaled by c_j = e^{+r_j/2}; the scale is
    constant per column so it factors through P = K M and is cancelled
    exactly in stage V by MX = Mp . e^{-r/2}:
        PT  = Mp^T K''   (9-column weight loads, f32r stream, two PSUM
                          accumulation groups that chase the exp stream)
        P'' = PT^T per 128-col block (8 small PE transposes, one PSUM bank)
        V   = MX^T P'' = Mp^T K Mp
  - K symmetry makes the stored K'' row-tiles serve both orientations, so
    the [n,n] matrix is never transposed.
"""

import sys

sys.path.insert(0, "/opt/trn_rl_repo")

import numpy as np

_B, _N, _D = 8, 1024, 4
_P = 128
_NT = _N // _P  # 8 row tiles
_NM = 1 + 2 * _D  # 9 basis columns: [1, p, p^2]
_INV_SQRT_2PI = 1.0 / np.sqrt(2.0 * np.pi)
_RK = 0.282095
_FACT = 1.0592 * float(_N) ** (-1.0 / (4 + _D))

_NC = None  # compiled Bass module cache


def _build_kernel():
    import concourse.bass as bass  # noqa: F401
    import concourse.tile as tile
    from concourse import bacc, mybir
    from concourse.masks import make_identity

    f32 = mybir.dt.float32
    fr = mybir.dt.float32r
    Act = mybir.ActivationFunctionType
    Alu = mybir.AluOpType
    Ax = mybir.AxisListType

    nc = bacc.Bacc("TRN2", target_bir_lowering=False, debug=False, num_devices=_B)
    p_in = nc.dram_tensor("p", [_N, _D], f32, kind="ExternalInput")
    v_out = nc.dram_tensor("vout", [_NM, _NM], f32, kind="ExternalOutput")
    var_out = nc.dram_tensor("varout", [_D, 1], f32, kind="ExternalOutput")

    with tile.TileContext(nc) as tc:
        with (
            tc.tile_pool(name="singles", bufs=1) as singles,
            tc.tile_pool(name="psE", bufs=1, space="PSUM") as psE,
            tc.tile_pool(name="psV", bufs=1, space="PSUM") as psV,
            tc.tile_pool(name="psG", bufs=2, space="PSUM") as psG,
            tc.tile_pool(name="psPT", bufs=1, space="PSUM") as psPT,
        ):
            ident128 = singles.tile([_P, _P], f32, tag="identf")
            make_identity(nc, ident128)
            ident = ident128[0:_NM, 0:_NM]
            ones128 = singles.tile([_P, 1], f32, tag="ones128")
            nc.vector.memset(ones128, 1.0)
            ones_row = singles.tile([1, _P], f32, tag="ones_row")
            nc.vector.memset(ones_row, 1.0)
            onesN = singles.tile([_P, 1], f32, tag="onesN")
            nc.vector.memset(onesN, 1.0 / float(_N) ** 0.5)
            # dummy Exp so the activation-table load runs during the DMA wait
            warm = singles.tile([1, 1], f32, tag="warm")
            nc.scalar.activation(out=warm, in_=ones128[0:1, 0:1], func=Act.Exp)

            # ---- two input DMAs: particle-major tiles + feature-major rows
            mstatall = singles.tile([_P, _NT, _D], f32, tag="mstatall")
            nc.sync.dma_start(
                out=mstatall, in_=p_in[:].rearrange("(c i) d -> i c d", c=_NT)
            )
            msqall = singles.tile([_P, _NT, _D], f32, tag="msqall")
            nc.vector.tensor_mul(msqall, mstatall, mstatall)

            # ---- stats on the PE: two sequential accumulation groups
            # (sum p, then sum p^2), each copied out and transposed to a
            # row so the var chain runs at partition 0
            sv4 = []
            for g, (src, rv) in enumerate(((mstatall, onesN), (msqall, ones128))):
                st4 = psE.tile([_D, 1], f32, tag="early")
                for c in range(_NT):
                    nc.tensor.matmul(
                        st4, lhsT=src[:, c, :], rhs=rv,
                        start=(c == 0), stop=(c == _NT - 1),
                    )
                sv = singles.tile([_D, 1], f32, tag=f"sv4_{g}")
                nc.vector.tensor_copy(sv, st4)
                sv4.append(sv)
            # den = sum(p^2) - sum(p)^2/n = (n-1) var; phcol = 0.5/pilot^2
            den = singles.tile([_D, 1], f32, tag="den")
            nc.vector.tensor_mul(den, sv4[0], sv4[0])
            nc.vector.tensor_sub(den, sv4[1], den)
            var_t = singles.tile([_D, 1], f32, tag="var_t")
            nc.vector.tensor_scalar_mul(var_t, den, 1.0 / (_N - 1))
            nc.sync.dma_start(out=var_out[:], in_=var_t)
            denf = singles.tile([_D, 1], f32, tag="denf")
            nc.vector.tensor_scalar_mul(
                denf, den, 2.0 * _FACT * _FACT / (_N - 1)
            )
            phcol = singles.tile([_D, 1], f32, tag="phcol")
            nc.vector.reciprocal(phcol, denf)

            # QTr = p in feature-major f32r via 8 PE transposes of the
            # tile-major data (no second DMA); Qs = QTr * 2*phcol
            QTr = singles.tile([_D, _N], fr, tag="qtr")
            for c in range(_NT):
                cs = slice(c * _P, (c + 1) * _P)
                ps_q = psG.tile([_D, _P], f32, tag="psg")
                nc.tensor.transpose(ps_q, mstatall[:, c, :], ident128)
                nc.vector.tensor_copy(QTr[:, cs], ps_q)
            Qs = singles.tile([_D, _N], fr, tag="qs")
            nc.vector.tensor_scalar(
                out=Qs, in0=QTr, scalar1=phcol, scalar2=2.0,
                op0=Alu.mult, op1=Alu.mult,
            )

            # 0.5/pilot^2 as a row + broadcast to [128,4] via rank-1 PE
            # outer product (for the r_i reductions)
            ps_pr = psE.tile([1, _D], f32, tag="early")
            nc.tensor.transpose(ps_pr, phcol, ident[0:_D, 0:_D])
            ph_r = singles.tile([1, _D], f32, tag="ph_r")
            nc.vector.tensor_copy(ph_r, ps_pr)
            ps_bc = psE.tile([_P, _D], f32, tag="early")
            nc.tensor.matmul(ps_bc, lhsT=ones_row, rhs=ph_r, start=True, stop=True)
            bc_sb = singles.tile([_P, _D], f32, tag="bc_sb")
            nc.vector.tensor_copy(bc_sb, ps_bc)

            # ---- exp bias nhall[:, c] = -r/2 = -sum_d p^2 * (0.5/pilot^2)
            nhall = singles.tile([_P, _NT], f32, tag="nhall")
            scr = singles.tile([_P, _NT, _D], f32, tag="scr")
            for c in range(_NT):
                nc.vector.tensor_mul(scr[:, c, :], msqall[:, c, :], bc_sb)
                nc.vector.tensor_reduce(
                    out=nhall[:, c : c + 1], in_=scr[:, c, :],
                    axis=Ax.X, op=Alu.add, negate=True,
                )

            # ---- Mp tiles (f32r, PT-stage weights) built in strided copies;
            # MX = Mp . e^{-r/2} per tile
            mtall = singles.tile([_P, _NT, _NM], fr, tag="mtall")
            for c in range(_NT):
                nc.vector.tensor_copy(mtall[:, c, 0:1], ones128)
            nc.vector.tensor_copy(mtall[:, :, 1 : 1 + _D], mstatall)
            nc.vector.tensor_copy(mtall[:, :, 1 + _D : _NM], msqall)
            cneg = singles.tile([_P, _NT], f32, tag="cneg")
            nc.scalar.activation(out=cneg, in_=nhall, func=Act.Exp)
            mxall = singles.tile([_P, _NT, _NM], f32, tag="mxall")
            for c in range(_NT):
                nc.vector.tensor_scalar_mul(
                    mxall[:, c, :], mtall[:, c, :], cneg[:, c : c + 1]
                )

            # ---- main stream: per row tile, two f32r Gram matmuls into a
            # 2-bank PSUM tile, one [128,1024] Exp, then the tile's PT
            # contributions (both j-half accumulation groups chase the exps)
            KT = singles.tile([_P, _NT, _N], fr, tag="kt")
            pspt = psPT.tile([_NM, 2, 512], f32, tag="pspt")
            for ir in range(_NT):
                irs = slice(ir * _P, (ir + 1) * _P)
                psg = psG.tile([_P, 2, 512], f32, tag="psg")
                for jh in range(2):
                    js = slice(jh * 512, (jh + 1) * 512)
                    nc.tensor.matmul(
                        psg[:, jh, :],
                        lhsT=Qs[:, irs],
                        rhs=QTr[:, js],
                        start=True, stop=True,
                    )
                nc.scalar.activation(
                    out=KT[:, ir, :],
                    in_=psg.rearrange("p a b -> p (a b)"),
                    func=Act.Exp,
                    bias=nhall[:, ir : ir + 1],
                )
                for jh in range(2):
                    js = slice(jh * 512, (jh + 1) * 512)
                    nc.tensor.matmul(
                        pspt[:, jh, :],
                        lhsT=mtall[:, ir, :],
                        rhs=KT[:, ir, js],
                        start=(ir == 0), stop=(ir == _NT - 1),
                        skip_group_check=True,
                    )

            # ---- PT out of PSUM, P'' = PT^T per block into one PSUM bank,
            # V = MX^T P''
            pts = singles.tile([_NM, _N], f32, tag="pts")
            nc.vector.tensor_copy(pts[:, 0:512], pspt[:, 0, :])
            nc.vector.tensor_copy(pts[:, 512:1024], pspt[:, 1, :])
            psp2 = psE.tile([_P, _NT, _NM], f32, tag="early")
            for r in range(_NT):
                nc.tensor.transpose(
                    psp2[:, r, :], pts[:, r * _P : (r + 1) * _P], ident
                )
            prall = singles.tile([_P, _NT, _NM], f32, tag="prall")
            nc.vector.tensor_copy(prall, psp2)
            psv = psV.tile([_NM, _NM], f32, tag="psv")
            for r in range(_NT):
                nc.tensor.matmul(
                    psv, lhsT=mxall[:, r, :], rhs=prall[:, r, :],
                    start=(r == 0), stop=(r == _NT - 1),
                )
            Vt = singles.tile([_NM, _NM], f32, tag="vt")
            nc.vector.tensor_copy(Vt, psv)
            nc.sync.dma_start(out=v_out[:], in_=Vt)

    nc.compile()
    return nc


def _get_nc():
    global _NC
    if _NC is None:
        _NC = _build_kernel()
    return _NC


def finalize(V, var):
    """Host-side tail: V [9,9] (raw-p units), var [4] -> bandwidth [4]."""
    V = V.astype(np.float64)
    var = var.astype(np.float64).reshape(_D)
    pilot = _FACT * np.sqrt(var)
    d = np.arange(_D)
    s2 = (
        (V[0, 5 + d] + V[5 + d, 0] - 2.0 * V[1 + d, 1 + d]) / pilot**2 - V[0, 0]
    ) * _INV_SQRT_2PI
    denom = _N * (_N - 1)
    I2 = s2 / pilot**5 / denom
    J1 = _RK / I2
    base = J1 / _N
    return (np.sign(base) * np.abs(base) ** 0.2).astype(np.float32)


def kernel(particles, weights=None, **_unused):
    from concourse.bass_utils import run_bass_kernel_spmd

    particles = np.ascontiguousarray(np.asarray(particles), dtype=np.float32)
    assert particles.shape == (_B, _N, _D), particles.shape

    nc = _get_nc()
    in_maps = [{"p": particles[c]} for c in range(_B)]
    res = run_bass_kernel_spmd(nc, in_maps, list(range(_B)))

    out = np.empty((_B, _D), np.float32)
    for c in range(_B):
        out[c] = finalize(res.results[c]["vout"], res.results[c]["varout"])
    return out



# revision 9
# speedup vs baseline: 1.2769x; 1.2769x over previous
"""Trainium2 Bass kernel for nn_BandwidthPredictorNNHall.

Math: for each batch b (8 of them, one per NeuronCore) with particles
x [n=1024, d=4]:
    pilot_d = 1.0592 * std(x_d, ddof=1) * n^(-1/8)
    q = x / pilot,   K_ij = exp(-0.5 * |q_i - q_j|^2)
    s2_d = sum_ij K_ij ((q_jd - q_id)^2 - 1)
    s3 terms are exactly 0 by antisymmetry (treated as 0; fp noise in the
    reference, |bw2/bw1| ~ 6e-9).
With Mp = [1, p_1..p_4, p_1^2..p_4^2] (n x 9, RAW particle units), every sum
needed for s2 is an entry of V = Mp^T K Mp:
    s2_d = ((V[0,5+d] + V[5+d,0] - 2 V[1+d,1+d]) / pilot_d^2 - V[0,0]) / sqrt(2pi)
The device computes Vu/Vd (9x9 each) and var (4) per batch; the host forms
V = Vu + Vu^T - Vd (K-symmetry: upper-block + its transpose - diagonal
blocks counted twice) and applies the final ~30 scalar flops per batch.

Device pipeline per core (latency-driven; ScalarE's exps are the floor):
  - One input DMA in a partition-contiguous layout (partition p holds the 8
    consecutive particles 8p..8p+7 as "tiles" c=0..7: 128 descriptors of
    128B instead of 1024 x 16B). All downstream math is permutation-
    invariant as long as the particle enumeration j = c*128 + p is used
    consistently on both sides of K.
  - Stats run on the PE in BOTH orientations (row [1,4] for the
    nhall/broadcast chain, col [4,1] for the per-partition Q scaling) so
    neither orientation needs a transpose of the other.
  - K'' = exp(G - r_i/2) row tiles, UPPER TRIANGLE ONLY: tile ir covers
    columns j >= 128*ir (4608 exp columns instead of 8192).
    K'' = K * e^{+r_j/2}; the column scale cancels in stage V.
  - P-stage contracts over i directly: psPs[jb] += KT_chunk(lhsT) @ Mp_tile
    (9-column outputs ~15ns each on the PE; weight loads are free), plus a
    diagonal-only accumulator psPd[jb] from the jb==ir block.
  - Vu = sum_c MX[c]^T PsbS[c], Vd = sum_c MX[c]^T PsbD[c]  (MX = Mp e^{-r/2}
    cancels the column scale), one [9,2,9] copy, one output DMA.
"""

import sys

sys.path.insert(0, "/opt/trn_rl_repo")

import numpy as np

_B, _N, _D = 8, 1024, 4
_P = 128
_NT = _N // _P  # 8 column/row tiles
_NM = 1 + 2 * _D  # 9 basis columns: [1, p, p^2]
_INV_SQRT_2PI = 1.0 / np.sqrt(2.0 * np.pi)
_RK = 0.282095
_FACT = 1.0592 * float(_N) ** (-1.0 / (4 + _D))

_NC = None  # compiled Bass module cache


def _build_kernel():
    import concourse.bass as bass  # noqa: F401
    import concourse.tile as tile
    from concourse import bacc, mybir
    from concourse.masks import make_identity

    f32 = mybir.dt.float32
    fr = mybir.dt.float32r
    Act = mybir.ActivationFunctionType
    Alu = mybir.AluOpType
    Ax = mybir.AxisListType

    nc = bacc.Bacc("TRN2", target_bir_lowering=False, debug=False, num_devices=_B)
    p_in = nc.dram_tensor("p", [_N, _D], f32, kind="ExternalInput")
    v_out = nc.dram_tensor("vout", [_NM, 2 * _NM], f32, kind="ExternalOutput")
    var_out = nc.dram_tensor("varout", [1, _D], f32, kind="ExternalOutput")

    with tile.TileContext(nc) as tc:
        with (
            tc.tile_pool(name="singles", bufs=1) as singles,
            tc.tile_pool(name="psE", bufs=1, space="PSUM") as psE,
            tc.tile_pool(name="psA", bufs=1, space="PSUM") as psA,
            tc.tile_pool(name="psG", bufs=2, space="PSUM") as psG,
            tc.tile_pool(name="psT", bufs=2, space="PSUM") as psT,
        ):
            # ---- input DMA first in SP program order (data-ready gates all)
            mstatall = singles.tile([_P, _NT, _D], f32, tag="mstatall")
            nc.sync.dma_start(
                out=mstatall, in_=p_in[:].rearrange("(i c) d -> i c d", i=_P)
            )

            # dummy Exp so the activation-table load runs during the DMA wait
            warm = singles.tile([1, 1], f32, tag="warm")

            ident128 = singles.tile([_P, _P], f32, tag="identf")
            make_identity(nc, ident128)
            ones128 = singles.tile([_P, 1], f32, tag="ones128")
            nc.gpsimd.memset(ones128, 1.0)
            onesN = singles.tile([_P, 1], f32, tag="onesN")
            nc.gpsimd.memset(onesN, 1.0 / float(_N) ** 0.5)
            nc.scalar.activation(out=warm, in_=ident128[0:1, 0:1], func=Act.Exp)

            msqall = singles.tile([_P, _NT, _D], f32, tag="msqall")
            nc.vector.tensor_mul(msqall, mstatall, mstatall)

            # ---- stats on the PE, both orientations, all in one PSUM bank:
            #  row sums at early[0:1, 4:8] (p) and [0:1, 8:12] (p^2)
            #  col sums at early[0:4, 12:13] (p) and [0:4, 13:14] (p^2)
            early = psE.tile([_P, 16], f32, tag="early")
            for c in range(_NT):
                nc.tensor.matmul(
                    early[0:1, 4:8], lhsT=onesN, rhs=mstatall[:, c, :],
                    start=(c == 0), stop=(c == _NT - 1), skip_group_check=True,
                )
            for c in range(_NT):
                nc.tensor.matmul(
                    early[0:4, 12:13], lhsT=mstatall[:, c, :], rhs=onesN,
                    start=(c == 0), stop=(c == _NT - 1), skip_group_check=True,
                )
            for c in range(_NT):
                nc.tensor.matmul(
                    early[0:1, 8:12], lhsT=ones128, rhs=msqall[:, c, :],
                    start=(c == 0), stop=(c == _NT - 1), skip_group_check=True,
                )
            for c in range(_NT):
                nc.tensor.matmul(
                    early[0:4, 13:14], lhsT=msqall[:, c, :], rhs=ones128,
                    start=(c == 0), stop=(c == _NT - 1), skip_group_check=True,
                )

            # ---- 8 PE transposes -> QTr (feature-major f32r), copies on
            # DVE / ScalarE (gpsimd cannot read PSUM)
            QTr = singles.tile([_D, _N], fr, tag="qtr")
            for c in range(_NT):
                ps_q = psT.tile([_D, _P], f32, tag="pst", name=f"psq{c}")
                nc.tensor.transpose(ps_q, mstatall[:, c, :], ident128)
                cs = slice(c * _P, (c + 1) * _P)
                if c % 2 == 0:
                    nc.vector.tensor_copy(QTr[:, cs], ps_q)
                else:
                    nc.scalar.activation(out=QTr[:, cs], in_=ps_q, func=Act.Copy)

            # ---- var chains on DVE (row form feeds nhall; col form feeds
            # the per-partition Q scaling)
            svr = singles.tile([1, 8], f32, tag="svr")
            nc.vector.tensor_copy(svr, early[0:1, 4:12])
            svc = singles.tile([_D, 2], f32, tag="svc")
            nc.vector.tensor_copy(svc, early[0:4, 12:14])

            # row chain: den_r = sump2 - sump^2 ( = (n-1) var ), ph_row =
            # 0.5/pilot^2
            den_r = singles.tile([1, _D], f32, tag="den_r")
            nc.vector.tensor_mul(den_r, svr[:, 0:4], svr[:, 0:4])
            nc.vector.tensor_sub(den_r, svr[:, 4:8], den_r)
            var_t = singles.tile([1, _D], f32, tag="var_t")
            nc.vector.tensor_scalar_mul(var_t, den_r, 1.0 / (_N - 1))
            nc.sync.dma_start(out=var_out[:], in_=var_t)
            denf_r = singles.tile([1, _D], f32, tag="denf_r")
            nc.vector.tensor_scalar_mul(denf_r, den_r, 2.0 * _FACT * _FACT / (_N - 1))
            ph_row = singles.tile([1, _D], f32, tag="ph_row")
            nc.vector.reciprocal(ph_row, denf_r)

            # col chain (1/pilot^2, [4,1] partition-major, feeds qs_t scale)
            den_c = singles.tile([_D, 1], f32, tag="den_c")
            nc.vector.tensor_mul(den_c, svc[:, 0:1], svc[:, 0:1])
            nc.vector.tensor_sub(den_c, svc[:, 1:2], den_c)
            denf_c = singles.tile([_D, 1], f32, tag="denf_c")
            nc.vector.tensor_scalar_mul(denf_c, den_c, _FACT * _FACT / (_N - 1))
            phcol = singles.tile([_D, 1], f32, tag="phcol")
            nc.vector.reciprocal(phcol, denf_c)

            # bc_sb[128, 4] = ph_row broadcast to all partitions (gpsimd)
            bc_sb = singles.tile([_P, _D], f32, tag="bc_sb")
            nc.gpsimd.partition_broadcast(bc_sb, ph_row)

            # ---- qs_t tiles: q-scaled lhsT rows, per tile (DVE/gpsimd)
            qs_t = singles.tile([_D, _NT, _P], fr, tag="qs_t")
            for c in range(_NT):
                cs = slice(c * _P, (c + 1) * _P)
                eng = (nc.vector, nc.gpsimd)[c % 2]
                eng.tensor_scalar_mul(qs_t[:, c, :], QTr[:, cs], phcol)

            # ---- exp bias nhall[:, c] = -r/2 = -sum_d p^2 * (0.5/pilot^2)
            nhall = singles.tile([_P, _NT], f32, tag="nhall")
            scr = singles.tile([_P, _NT, _D], f32, tag="scr")
            for c in range(_NT):
                eng = (nc.vector, nc.gpsimd)[c % 2]
                eng.tensor_mul(scr[:, c, :], msqall[:, c, :], bc_sb)
                nc.vector.tensor_reduce(
                    out=nhall[:, c : c + 1], in_=scr[:, c, :],
                    axis=Ax.X, op=Alu.add, negate=True,
                )

            # ---- Mp tiles (f32r): [1, p, p^2]
            mtall = singles.tile([_P, _NT, _NM], f32, tag="mtall")
            for c in range(_NT):
                nc.gpsimd.tensor_copy(mtall[:, c, 0:1], ones128)
            nc.gpsimd.tensor_copy(mtall[:, :, 1 : 1 + _D], mstatall)
            nc.gpsimd.tensor_copy(mtall[:, :, 1 + _D : _NM], msqall)

            # ---- PSUM accumulators in one bank:
            # psPs (8 upper chunks), psPd (8 diag-only chunks), psVu, psVd
            accT = psA.tile([_P, 18, _NM], f32, tag="accT")
            # start=True on any matmul would mark the whole 2KB zero-region
            # (bank) pending-zero and wipe the other accumulators sharing it;
            # zero once, then accumulate-only (start=False everywhere).
            nc.vector.memset(accT, 0.0)
            psPs = accT[:, 0:8, :]
            psPd = accT[:, 8:16, :]
            psVu = accT[0:_NM, 16, :]
            psVd = accT[0:_NM, 17, :]

            KT = singles.tile([_P, _NT, _N], f32, tag="kt")
            cneg = singles.tile([_P, _NT], f32, tag="cneg")
            mxall = singles.tile([_P, _NT, _NM], f32, tag="mxall")

            psgs = [None, None]

            def emit_gram(ir):
                """Gram for row tile ir over columns [128*ir, 1024)."""
                psg = psG.tile([_P, _N], f32, tag="psg", name=f"psg{ir}")
                psgs[ir % 2] = psg
                pieces = [(0, 512), (512, 1024)]
                for a, b in pieces:
                    nc.tensor.matmul(
                        psg[:, a:b], lhsT=qs_t[:, ir, :], rhs=QTr[:, a:b],
                        start=True, stop=True,
                    )
                return psg

            def emit_exp(ir, psg):
                s = ir * _P
                nc.scalar.activation(
                    out=KT[:, ir, s:_N], in_=psg[:, s:_N],
                    func=Act.Exp, bias=nhall[:, ir : ir + 1],
                )

            def emit_pdirect(ir):
                for jb in range(ir, _NT):
                    nc.tensor.matmul(
                        psPs[:, jb, :],
                        lhsT=KT[:, ir, jb * _P : (jb + 1) * _P],
                        rhs=mtall[:, ir, :],
                        start=False, stop=(ir == jb),
                        skip_group_check=True,
                    )
                # diagonal-only accumulator (same operands, jb == ir)
                nc.tensor.matmul(
                    psPd[:, ir, :],
                    lhsT=KT[:, ir, ir * _P : (ir + 1) * _P],
                    rhs=mtall[:, ir, :],
                    start=False, stop=True,
                    skip_group_check=True,
                )

            # ---- main triangle loop, software-pipelined emission
            for ir in range(_NT):
                psg = emit_gram(ir)
                emit_exp(ir, psg)
                if ir == 1:
                    # cneg/mxall right after exp0 on ScalarE; needed only at
                    # the V stage
                    nc.scalar.activation(out=cneg, in_=nhall, func=Act.Exp)
                    for c in range(_NT):
                        eng = (nc.vector, nc.gpsimd)[c % 2]
                        eng.tensor_scalar_mul(mxall[:, c, :], mtall[:, c, :],
                                              cneg[:, c : c + 1])
                if ir >= 1:
                    emit_pdirect(ir - 1)
            emit_pdirect(_NT - 1)

            # ---- drain accumulators, V matmuls, output
            PsbS = singles.tile([_P, 8, _NM], f32, tag="psbs")
            nc.vector.tensor_copy(PsbS, psPs)
            PsbD = singles.tile([_P, 8, _NM], f32, tag="psbd")
            nc.scalar.activation(out=PsbD, in_=psPd, func=Act.Copy)

            for c in range(_NT):
                nc.tensor.matmul(
                    psVu, lhsT=mxall[:, c, :], rhs=PsbS[:, c, :],
                    start=False, stop=(c == _NT - 1), skip_group_check=True,
                )
            for c in range(_NT):
                nc.tensor.matmul(
                    psVd, lhsT=mxall[:, c, :], rhs=PsbD[:, c, :],
                    start=False, stop=(c == _NT - 1), skip_group_check=True,
                )

            Vt = singles.tile([_NM, 2, _NM], f32, tag="vt")
            nc.vector.tensor_copy(Vt, accT[0:_NM, 16:18, :])
            nc.sync.dma_start(
                out=v_out[:].rearrange("a (b c) -> a b c", b=2), in_=Vt
            )

    nc.compile()
    return nc


def _get_nc():
    global _NC
    if _NC is None:
        _NC = _build_kernel()
    return _NC


def finalize(Vud, var):
    """Host-side tail: Vud [9, 2*9] = [Vu | Vd] interleaved as [9,2,9]
    (raw-p units), var [4] -> bandwidth [4]."""
    Vud = Vud.astype(np.float64).reshape(_NM, 2, _NM)
    Vu = Vud[:, 0, :]
    Vd = Vud[:, 1, :]
    V = Vu + Vu.T - Vd
    var = var.astype(np.float64).reshape(_D)
    pilot = _FACT * np.sqrt(var)
    d = np.arange(_D)
    s2 = (
        (V[0, 5 + d] + V[5 + d, 0] - 2.0 * V[1 + d, 1 + d]) / pilot**2 - V[0, 0]
    ) * _INV_SQRT_2PI
    denom = _N * (_N - 1)
    I2 = s2 / pilot**5 / denom
    J1 = _RK / I2
    base = J1 / _N
    return (np.sign(base) * np.abs(base) ** 0.2).astype(np.float32)


def kernel(particles, weights=None, **_unused):
    from concourse.bass_utils import run_bass_kernel_spmd

    particles = np.ascontiguousarray(np.asarray(particles), dtype=np.float32)
    assert particles.shape == (_B, _N, _D), particles.shape

    nc = _get_nc()
    in_maps = [{"p": particles[c]} for c in range(_B)]
    res = run_bass_kernel_spmd(nc, in_maps, list(range(_B)))

    out = np.empty((_B, _D), np.float32)
    for c in range(_B):
        out[c] = finalize(res.results[c]["vout"], res.results[c]["varout"])
    return out


# revision 10
# speedup vs baseline: 1.2916x; 1.0115x over previous
"""Trainium2 Bass kernel for nn_BandwidthPredictorNNHall.

Math: for each batch b (8 of them, one per NeuronCore) with particles
x [n=1024, d=4]:
    pilot_d = 1.0592 * std(x_d, ddof=1) * n^(-1/8)
    q = x / pilot,   K_ij = exp(-0.5 * |q_i - q_j|^2)
    s2_d = sum_ij K_ij ((q_jd - q_id)^2 - 1)
    s3 terms are exactly 0 by antisymmetry (treated as 0; fp noise in the
    reference, |bw2/bw1| ~ 6e-9).
With Mp = [1, p_1..p_4, p_1^2..p_4^2] (n x 9, RAW particle units), every sum
needed for s2 is an entry of V = Mp^T K Mp:
    s2_d = ((V[0,5+d] + V[5+d,0] - 2 V[1+d,1+d]) / pilot_d^2 - V[0,0]) / sqrt(2pi)
The device computes Vu/Vd (9x9 each) and var (4) per batch; the host forms
V = Vu + Vu^T - Vd (K-symmetry: upper-block + its transpose - diagonal
blocks counted twice) and applies the final ~30 scalar flops per batch.

Device pipeline per core (latency-driven; ScalarE's exps are the floor):
  - One input DMA in a partition-contiguous layout (partition p holds the 8
    consecutive particles 8p..8p+7 as "tiles" c=0..7: 128 descriptors of
    128B instead of 1024 x 16B). All downstream math is permutation-
    invariant as long as the particle enumeration j = c*128 + p is used
    consistently on both sides of K.
  - Stats run on the PE in BOTH orientations (row [1,4] for the
    nhall/broadcast chain, col [4,1] for the per-partition Q scaling) so
    neither orientation needs a transpose of the other.
  - K'' = exp(G - r_i/2) row tiles, UPPER TRIANGLE ONLY: tile ir covers
    columns j >= 128*ir (4608 exp columns instead of 8192).
    K'' = K * e^{+r_j/2}; the column scale cancels in stage V.
  - P-stage contracts over i directly: psPs[jb] += KT_chunk(lhsT) @ Mp_tile
    (9-column outputs ~15ns each on the PE; weight loads are free), plus a
    diagonal-only accumulator psPd[jb] from the jb==ir block.
  - Vu = sum_c MX[c]^T PsbS[c], Vd = sum_c MX[c]^T PsbD[c]  (MX = Mp e^{-r/2}
    cancels the column scale), one [9,2,9] copy, one output DMA.
"""

import sys

sys.path.insert(0, "/opt/trn_rl_repo")

import numpy as np

_B, _N, _D = 8, 1024, 4
_P = 128
_NT = _N // _P  # 8 column/row tiles
_NM = 1 + 2 * _D  # 9 basis columns: [1, p, p^2]
_INV_SQRT_2PI = 1.0 / np.sqrt(2.0 * np.pi)
_RK = 0.282095
_FACT = 1.0592 * float(_N) ** (-1.0 / (4 + _D))

_NC = None  # compiled Bass module cache


def _build_kernel():
    import concourse.bass as bass  # noqa: F401
    import concourse.tile as tile
    from concourse import bacc, mybir
    from concourse.masks import make_identity

    f32 = mybir.dt.float32
    fr = mybir.dt.float32r
    Act = mybir.ActivationFunctionType
    Alu = mybir.AluOpType
    Ax = mybir.AxisListType

    nc = bacc.Bacc("TRN2", target_bir_lowering=False, debug=False, num_devices=_B)
    p_in = nc.dram_tensor("p", [_N, _D], f32, kind="ExternalInput")
    v_out = nc.dram_tensor("vout", [_NM, 2 * _NM], f32, kind="ExternalOutput")
    var_out = nc.dram_tensor("varout", [1, _D], f32, kind="ExternalOutput")

    with tile.TileContext(nc) as tc:
        with (
            tc.tile_pool(name="singles", bufs=1) as singles,
            tc.tile_pool(name="psE", bufs=1, space="PSUM") as psE,
            tc.tile_pool(name="psA", bufs=1, space="PSUM") as psA,
            tc.tile_pool(name="psG", bufs=2, space="PSUM") as psG,
            tc.tile_pool(name="psT", bufs=2, space="PSUM") as psT,
        ):
            # ---- input DMA first in SP program order (data-ready gates all)
            mstatall = singles.tile([_P, _NT, _D], f32, tag="mstatall")
            nc.sync.dma_start(
                out=mstatall, in_=p_in[:].rearrange("(i c) d -> i c d", i=_P)
            )

            # dummy Exp so the activation-table load runs during the DMA wait
            warm = singles.tile([1, 1], f32, tag="warm")

            ident128 = singles.tile([_P, _P], f32, tag="identf")
            make_identity(nc, ident128)
            ones128 = singles.tile([_P, 1], f32, tag="ones128")
            nc.gpsimd.memset(ones128, 1.0)
            onesN = singles.tile([_P, 1], f32, tag="onesN")
            nc.gpsimd.memset(onesN, 1.0 / float(_N) ** 0.5)
            nc.scalar.activation(out=warm, in_=ident128[0:1, 0:1], func=Act.Exp)

            msqall = singles.tile([_P, _NT, _D], f32, tag="msqall")
            nc.vector.tensor_mul(msqall, mstatall, mstatall)

            # ---- stats on the PE, both orientations, all in one PSUM bank:
            #  row sums at early[0:1, 4:8] (p) and [0:1, 8:12] (p^2)
            #  col sums at early[0:4, 12:13] (p) and [0:4, 13:14] (p^2)
            early = psE.tile([_P, 16], f32, tag="early")
            for c in range(_NT):
                nc.tensor.matmul(
                    early[0:1, 4:8], lhsT=onesN, rhs=mstatall[:, c, :],
                    start=(c == 0), stop=(c == _NT - 1), skip_group_check=True,
                )
            for c in range(_NT):
                nc.tensor.matmul(
                    early[0:4, 12:13], lhsT=mstatall[:, c, :], rhs=onesN,
                    start=(c == 0), stop=(c == _NT - 1), skip_group_check=True,
                )
            for c in range(_NT):
                nc.tensor.matmul(
                    early[0:1, 8:12], lhsT=ones128, rhs=msqall[:, c, :],
                    start=(c == 0), stop=(c == _NT - 1), skip_group_check=True,
                )
            for c in range(_NT):
                nc.tensor.matmul(
                    early[0:4, 13:14], lhsT=msqall[:, c, :], rhs=ones128,
                    start=(c == 0), stop=(c == _NT - 1), skip_group_check=True,
                )

            # ---- 8 PE transposes -> QTr (feature-major f32r).  Four
            # transposes share one [4,512] PSUM quad (2KB zero region): the
            # first uses start=True (marks the whole bank pending-zero), the
            # rest start=False (their bytes zero on first touch), so one
            # 512-wide copy drains four tiles.  Two quads: DVE + ScalarE
            # copies run in parallel (gpsimd cannot read PSUM).
            QTr = singles.tile([_D, _N], fr, tag="qtr")
            for q in range(2):
                ps_q = psT.tile([_D, 4 * _P], f32, tag="pst", name=f"psq{q}")
                for k in range(4):
                    c = q * 4 + k
                    nc.tensor.matmul(
                        ps_q[:, k * _P : (k + 1) * _P],
                        lhsT=mstatall[:, c, :], rhs=ident128,
                        is_transpose=True, start=(k == 0), stop=True,
                        skip_group_check=True,
                    )
                cs = slice(q * 4 * _P, (q + 1) * 4 * _P)
                if q == 0:
                    nc.vector.tensor_copy(QTr[:, cs], ps_q)
                else:
                    nc.scalar.activation(out=QTr[:, cs], in_=ps_q, func=Act.Copy)

            # ---- var chains on DVE (row form feeds nhall; col form feeds
            # the per-partition Q scaling)
            svr = singles.tile([1, 8], f32, tag="svr")
            nc.vector.tensor_copy(svr, early[0:1, 4:12])
            svc = singles.tile([_D, 2], f32, tag="svc")
            nc.vector.tensor_copy(svc, early[0:4, 12:14])

            # row chain: den_r = sump2 - sump^2 ( = (n-1) var ), ph_row =
            # 0.5/pilot^2
            den_r = singles.tile([1, _D], f32, tag="den_r")
            nc.gpsimd.tensor_mul(den_r, svr[:, 0:4], svr[:, 0:4])
            nc.gpsimd.tensor_sub(den_r, svr[:, 4:8], den_r)
            var_t = singles.tile([1, _D], f32, tag="var_t")
            nc.gpsimd.tensor_scalar_mul(var_t, den_r, 1.0 / (_N - 1))
            nc.sync.dma_start(out=var_out[:], in_=var_t)
            denf_r = singles.tile([1, _D], f32, tag="denf_r")
            nc.gpsimd.tensor_scalar_mul(denf_r, den_r, 2.0 * _FACT * _FACT / (_N - 1))
            ph_row = singles.tile([1, _D], f32, tag="ph_row")
            nc.vector.reciprocal(ph_row, denf_r)

            # col chain (1/pilot^2, [4,1] partition-major, feeds qs_t scale)
            den_c = singles.tile([_D, 1], f32, tag="den_c")
            nc.gpsimd.tensor_mul(den_c, svc[:, 0:1], svc[:, 0:1])
            nc.gpsimd.tensor_sub(den_c, svc[:, 1:2], den_c)
            denf_c = singles.tile([_D, 1], f32, tag="denf_c")
            nc.gpsimd.tensor_scalar_mul(denf_c, den_c, _FACT * _FACT / (_N - 1))
            phcol = singles.tile([_D, 1], f32, tag="phcol")
            nc.vector.reciprocal(phcol, denf_c)

            # bc_sb[128, 4] = ph_row broadcast to all partitions (gpsimd)
            bc_sb = singles.tile([_P, _D], f32, tag="bc_sb")
            nc.gpsimd.partition_broadcast(bc_sb, ph_row)

            # ---- qs_t tiles (DVE) interleaved with the exp-bias chain:
            # nhall[:, c] = -r/2 = -sum_d p^2 * (0.5/pilot^2)
            # (scr muls on gpsimd, reduces on DVE)
            qs_t = singles.tile([_D, _NT, _P], fr, tag="qs_t")
            nhall = singles.tile([_P, _NT], f32, tag="nhall")
            scr = singles.tile([_P, _NT, _D], f32, tag="scr")
            for c in range(_NT):
                nc.gpsimd.tensor_mul(scr[:, c, :], msqall[:, c, :], bc_sb)
            for c in range(_NT):
                cs = slice(c * _P, (c + 1) * _P)
                nc.vector.tensor_scalar_mul(qs_t[:, c, :], QTr[:, cs], phcol)
                nc.vector.tensor_reduce(
                    out=nhall[:, c : c + 1], in_=scr[:, c, :],
                    axis=Ax.X, op=Alu.add, negate=True,
                )

            # ---- Mp tiles (f32r): [1, p, p^2]
            mtall = singles.tile([_P, _NT, _NM], f32, tag="mtall")
            for c in range(_NT):
                nc.gpsimd.tensor_copy(mtall[:, c, 0:1], ones128)
            nc.gpsimd.tensor_copy(mtall[:, :, 1 : 1 + _D], mstatall)
            nc.gpsimd.tensor_copy(mtall[:, :, 1 + _D : _NM], msqall)

            # ---- PSUM accumulators in one bank:
            # psPs (8 upper chunks), psPd (8 diag-only chunks), psVu, psVd
            accT = psA.tile([_P, 18, _NM], f32, tag="accT")
            # start=True on any matmul would mark the whole 2KB zero-region
            # (bank) pending-zero and wipe the other accumulators sharing it;
            # zero once, then accumulate-only (start=False everywhere).
            nc.vector.memset(accT, 0.0)
            psPs = accT[:, 0:8, :]
            psPd = accT[:, 8:16, :]
            psVu = accT[0:_NM, 16, :]
            psVd = accT[0:_NM, 17, :]

            KT = singles.tile([_P, _NT, _N], f32, tag="kt")
            cneg = singles.tile([_P, _NT], f32, tag="cneg")
            mxall = singles.tile([_P, _NT, _NM], f32, tag="mxall")

            psgs = [None, None]

            def emit_gram(ir):
                """Gram for row tile ir over columns [128*ir, 1024)."""
                psg = psG.tile([_P, _N], f32, tag="psg", name=f"psg{ir}")
                psgs[ir % 2] = psg
                pieces = [(0, 512), (512, 1024)]
                for a, b in pieces:
                    nc.tensor.matmul(
                        psg[:, a:b], lhsT=qs_t[:, ir, :], rhs=QTr[:, a:b],
                        start=True, stop=True,
                    )
                return psg

            def emit_exp(ir, psg):
                s = ir * _P
                nc.scalar.activation(
                    out=KT[:, ir, s:_N], in_=psg[:, s:_N],
                    func=Act.Exp, bias=nhall[:, ir : ir + 1],
                )

            def emit_pdirect(ir):
                for jb in range(ir, _NT):
                    nc.tensor.matmul(
                        psPs[:, jb, :],
                        lhsT=KT[:, ir, jb * _P : (jb + 1) * _P],
                        rhs=mtall[:, ir, :],
                        start=False, stop=(ir == jb),
                        skip_group_check=True,
                    )
                # diagonal-only accumulator (same operands, jb == ir)
                nc.tensor.matmul(
                    psPd[:, ir, :],
                    lhsT=KT[:, ir, ir * _P : (ir + 1) * _P],
                    rhs=mtall[:, ir, :],
                    start=False, stop=True,
                    skip_group_check=True,
                )

            PsbS = singles.tile([_P, 8, _NM], f32, tag="psbs")
            PsbD = singles.tile([_P, 8, _NM], f32, tag="psbd")

            def emit_drain(c):
                """Chunk c's psPs/psPd groups completed at iteration c."""
                nc.vector.tensor_copy(PsbS[:, c, :], psPs[:, c, :])
                nc.vector.tensor_copy(PsbD[:, c, :], psPd[:, c, :])

            def emit_v(c):
                nc.tensor.matmul(
                    psVu, lhsT=mxall[:, c, :], rhs=PsbS[:, c, :],
                    start=False, stop=(c == _NT - 1), skip_group_check=True,
                )
                nc.tensor.matmul(
                    psVd, lhsT=mxall[:, c, :], rhs=PsbD[:, c, :],
                    start=False, stop=(c == _NT - 1), skip_group_check=True,
                )

            # ---- main triangle loop, software-pipelined emission
            for ir in range(_NT):
                psg = emit_gram(ir)
                emit_exp(ir, psg)
                if ir == 1:
                    # cneg/mxall right after exp0 on ScalarE; needed only at
                    # the V stage
                    nc.scalar.activation(out=cneg, in_=nhall, func=Act.Exp)
                    for c in range(_NT):
                        nc.gpsimd.tensor_scalar_mul(
                            mxall[:, c, :], mtall[:, c, :], cneg[:, c : c + 1]
                        )
                if ir >= 1:
                    emit_pdirect(ir - 1)
                    emit_drain(ir - 1)
                if ir >= 2:
                    emit_v(ir - 2)
            emit_pdirect(_NT - 1)
            emit_drain(_NT - 1)
            emit_v(_NT - 2)
            emit_v(_NT - 1)

            Vt = singles.tile([_NM, 2, _NM], f32, tag="vt")
            nc.vector.tensor_copy(Vt, accT[0:_NM, 16:18, :])
            nc.sync.dma_start(
                out=v_out[:].rearrange("a (b c) -> a b c", b=2), in_=Vt
            )

    nc.compile()
    return nc


def _get_nc():
    global _NC
    if _NC is None:
        _NC = _build_kernel()
    return _NC


def finalize(Vud, var):
    """Host-side tail: Vud [9, 2*9] = [Vu | Vd] interleaved as [9,2,9]
    (raw-p units), var [4] -> bandwidth [4]."""
    Vud = Vud.astype(np.float64).reshape(_NM, 2, _NM)
    Vu = Vud[:, 0, :]
    Vd = Vud[:, 1, :]
    V = Vu + Vu.T - Vd
    var = var.astype(np.float64).reshape(_D)
    pilot = _FACT * np.sqrt(var)
    d = np.arange(_D)
    s2 = (
        (V[0, 5 + d] + V[5 + d, 0] - 2.0 * V[1 + d, 1 + d]) / pilot**2 - V[0, 0]
    ) * _INV_SQRT_2PI
    denom = _N * (_N - 1)
    I2 = s2 / pilot**5 / denom
    J1 = _RK / I2
    base = J1 / _N
    return (np.sign(base) * np.abs(base) ** 0.2).astype(np.float32)


def kernel(particles, weights=None, **_unused):
    from concourse.bass_utils import run_bass_kernel_spmd

    particles = np.ascontiguousarray(np.asarray(particles), dtype=np.float32)
    assert particles.shape == (_B, _N, _D), particles.shape

    nc = _get_nc()
    in_maps = [{"p": particles[c]} for c in range(_B)]
    res = run_bass_kernel_spmd(nc, in_maps, list(range(_B)))

    out = np.empty((_B, _D), np.float32)
    for c in range(_B):
        out[c] = finalize(res.results[c]["vout"], res.results[c]["varout"])
    return out
